# revision 27
# baseline (speedup 1.0000x reference)
"""Causal attention (B=16 heads, L=2048, D=64) on 8 TRN2 NeuronCores.

Sharding: head-parallel. Core i computes heads [2i, 2i+1] independently.

Per-head algorithm (all on one core, no collectives):
  For each 128-key chunk c:
    S^T[c] = matmul(lhsT=K^T[:, c], rhs=Q^T[:, q >= 128c])   # [128, nq] PSUM
    P^T[c] = exp(S^T[c] / 8)                                  # ACT, PSUM->SBUF
    mask strict-upper of diagonal block (affine_select)
    O'^T  += matmul(lhsT=[V_c | 1], rhs=P^T[c])               # [65, nq] PSUM acc
  O'^T row 64 holds the softmax denominators (ones-column trick).
  Tail: PSUM->SBUF, PE-transpose 128-col blocks, multiply by 1/denom, DMA out.

Q^T / K^T are built once per core with PE transposes, packing the two heads
into the 128 partitions (head0 in partitions 0:64, head1 in 64:128).
"""

import sys
from contextlib import ExitStack

sys.path.insert(0, "/opt/trn_rl_repo")

import numpy as np

import concourse.bass as bass
import concourse.mybir as mybir
import concourse.tile as tile
from concourse import bacc
from concourse.bass_utils import run_bass_kernel_spmd
from concourse.masks import make_identity

P = 128
L = 2048
D = 64
NB = L // P  # 16 key chunks / query blocks
H = 2  # heads per core
NCORES = 8
SW = 1024  # score-tile width (columns of S^T processed per exp op)

F32 = mybir.dt.float32
F32R = mybir.dt.float32r
BF16 = mybir.dt.bfloat16

# knobs: dtype of Q^T/K^T tiles (scores matmul) and of P^T/V tiles (output matmul)
QK_DT = BF16
PV_DT = BF16


def _mm(nc, out, lhsT, rhs, start, stop):
    nc.tensor.matmul(out, lhsT, rhs, start=start, stop=stop)


def build_body(ctx, nc, tc, q_ext, k_ext, v_ext, o_ext):
    Exp = mybir.ActivationFunctionType.Exp

    const = ctx.enter_context(tc.tile_pool(name="const", bufs=1))
    io = ctx.enter_context(tc.tile_pool(name="io", bufs=1))
    work = ctx.enter_context(tc.tile_pool(name="work", bufs=3))
    ps = ctx.enter_context(tc.tile_pool(name="ps", bufs=2, space="PSUM"))
    otp = ctx.enter_context(tc.tile_pool(name="otp", bufs=1, space="PSUM"))

    # ---- load inputs (heads interleaved along free dim for batched transpose)
    # half-granularity pipeline: load half -> cast half -> xbar-transpose half,
    # spread over both HWDGE queues and several compute engines
    assert QK_DT == BF16, "DMA transpose path requires 2-byte dtype"
    qn = io.tile([P, NB, H, D], F32)
    kn = io.tile([P, NB, H, D], F32)
    vst = io.tile([P, NB, H, D], F32)
    qnb = io.tile([P, NB, H, D], BF16)
    knb = io.tile([P, NB, H, D], BF16)
    qt = io.tile([P, NB, P], QK_DT)
    kt = io.tile([P, NB, P], QK_DT)
    v2 = io.tile([P, NB, H, D + 1], PV_DT)
    nc.vector.memset(v2[:], 1.0)  # ones everywhere; V data overwrites cols :D

    # queue discipline: scalar queue carries the q loads then ONLY xbar
    # transposes (a transpose<->copy mode switch serializes a queue); sync
    # carries plain loads/stores only.
    NH = NB // 2
    for m in range(2):
        ob = slice(m * NH, (m + 1) * NH)  # block-half
        for h in range(H):
            src = slice(m * NH * P, (m + 1) * NH * P)
            nc.scalar.dma_start(
                qn[:, ob, h, :], q_ext[h, src].rearrange("(o p) d -> p o d", p=P)
            )
            nc.sync.dma_start(
                kn[:, ob, h, :], k_ext[h, src].rearrange("(o p) d -> p o d", p=P)
            )
            nc.sync.dma_start(
                vst[:, ob, h, :], v_ext[h, src].rearrange("(o p) d -> p o d", p=P)
            )
        nc.vector.tensor_copy(knb[:, ob], kn[:, ob])
        nc.vector.tensor_copy(qnb[:, ob], qn[:, ob])
        nc.scalar.dma_start_transpose(kt[:, ob, :], knb[:, ob])
        nc.scalar.dma_start_transpose(qt[:, ob, :], qnb[:, ob])
        nc.gpsimd.tensor_copy(v2[:, ob, :, :D], vst[:, ob])

    # ---- per-head main loops
    for h in range(H):
        hp = slice(h * D, (h + 1) * D)  # partition range of this head in qt/kt
        ot = otp.tile([P, L], F32, tag="ot")  # O'^T accumulator, rows 0:65 used
        # otsb padded to 80 rows (xbar wants a multiple of 16); rows 65:80 are
        # garbage and land in otrs cols 65:80, which nothing reads.
        otsb = work.tile([80, L], BF16, tag="otsb", bufs=2)
        otrs = work.tile([P, NB, 80], BF16, tag="otrs", bufs=2)
        for c in range(NB):
            qoff = c * P
            # group start aligned to the 512 psum-bank grid so every matmul
            # output stays inside one bank of sps and of ot
            for g0 in range(qoff // 512 * 512, L, SW):
                lo_g = max(qoff, g0)
                w = min(SW, L - g0)  # group covers global cols [g0, g0+w)
                sps = ps.tile([P, SW], F32, tag="sps")
                j0 = lo_g
                while j0 < g0 + w:
                    j1 = min(j0 // 512 * 512 + 512, g0 + w)
                    _mm(
                        nc,
                        sps[:, j0 - g0 : j1 - g0],
                        lhsT=kt[hp, c, :],
                        rhs=qt[hp, j0 // P : j1 // P, :],
                        start=True,
                        stop=True,
                    )
                    j0 = j1
                pt = work.tile([P, SW], PV_DT, tag="pt", bufs=4)
                nc.scalar.activation(
                    pt[:, lo_g - g0 : w], sps[:, lo_g - g0 : w], Exp, scale=0.125
                )
                if lo_g == qoff:
                    # diagonal block: zero where q_local < k_partition
                    nc.gpsimd.affine_select(
                        out=pt[:, lo_g - g0 : lo_g - g0 + P],
                        in_=pt[:, lo_g - g0 : lo_g - g0 + P],
                        pattern=[[1, P]],
                        channel_multiplier=-1,
                        base=0,
                        compare_op=mybir.AluOpType.is_ge,
                        fill=0.0,
                    )
                j0 = lo_g
                while j0 < g0 + w:
                    j1 = min(j0 // 512 * 512 + 512, g0 + w)
                    b = j0 // 512
                    _mm(
                        nc,
                        ot[: D + 1, j0:j1],
                        lhsT=v2[:, c, h, :],
                        rhs=pt[:, j0 - g0 : j1 - g0],
                        start=(c == 0),
                        stop=(c == min(NB - 1, 4 * b + 3)),
                    )
                    j0 = j1
            if c % 4 == 3:
                # bank (c-3)//4 of ot got its last contribution — drain early
                b = (c - 3) // 4
                cols = slice(512 * b, 512 * (b + 1))
                nc.vector.tensor_copy(otsb[: D + 1, cols], ot[: D + 1, cols])

        # ---- tail, per 512-col quarter (deps let quarters b<3 run early)
        for b in range(4):
            cols = slice(512 * b, 512 * (b + 1))
            qb = slice(4 * b, 4 * (b + 1))
            nc.scalar.dma_start_transpose(otrs[:, qb, :], otsb[:, cols])
            rc = work.tile([P, 4], F32, tag="rc", bufs=4)
            nc.vector.reciprocal(rc, otrs[:, qb, D])
            osb = work.tile([P, 4, D], F32, tag="osb", bufs=4)
            nc.vector.tensor_tensor(
                osb,
                otrs[:, qb, :D],
                rc[:, :, None].to_broadcast((P, 4, D)),
                mybir.AluOpType.mult,
            )
            nc.sync.dma_start(o_ext[h, cols].rearrange("(o p) d -> p o d", p=P), osb)


_CACHE = {}

LDW_OPT = False
_PATCHED = False


def _patch_ldw_opt():
    global _PATCHED
    if _PATCHED or not LDW_OPT:
        return
    import concourse.bass_utils as bu

    orig = bu.run_command

    def run_command_ldw(cmd, *a, **kw):
        cmd = [
            "--enable-ldw-opt=true" if c == "--enable-ldw-opt=false" else c
            for c in cmd
        ]
        return orig(cmd, *a, **kw)

    bu.run_command = run_command_ldw
    _PATCHED = True


def _build():
    _patch_ldw_opt()
    nc = bacc.Bacc("TRN2", target_bir_lowering=False, debug=False, num_devices=NCORES)
    q_ext = nc.declare_dram_parameter("query", [H, L, D], F32, isOutput=False)
    k_ext = nc.declare_dram_parameter("key", [H, L, D], F32, isOutput=False)
    v_ext = nc.declare_dram_parameter("value", [H, L, D], F32, isOutput=False)
    o_ext = nc.declare_dram_parameter("out", [H, L, D], F32, isOutput=True)
    with tile.TileContext(nc) as tc, ExitStack() as ctx:
        build_body(ctx, nc, tc, q_ext, k_ext, v_ext, o_ext)
    nc.compile()
    return nc


def get_nc():
    key = (QK_DT, PV_DT, SW)
    if key not in _CACHE:
        _CACHE[key] = _build()
    return _CACHE[key]


def run(query, key, value, trace=False, tmpdir=None):
    query = np.ascontiguousarray(np.asarray(query, dtype=np.float32))
    key_ = np.ascontiguousarray(np.asarray(key, dtype=np.float32))
    value = np.ascontiguousarray(np.asarray(value, dtype=np.float32))
    nc = get_nc()
    in_maps = [
        {
            "query": query[H * i : H * (i + 1)],
            "key": key_[H * i : H * (i + 1)],
            "value": value[H * i : H * (i + 1)],
        }
        for i in range(NCORES)
    ]
    res = run_bass_kernel_spmd(
        nc, in_maps, core_ids=list(range(NCORES)), trace=trace, tmpdir=tmpdir
    )
    out = np.concatenate([res.results[i]["out"] for i in range(NCORES)], axis=0)
    return out.astype(np.float32), res


def kernel(query, key, value):
    out, _ = run(query, key, value, trace=False)
    return out
